# revision 1
# baseline (speedup 1.0000x reference)
"""PSROIPool Trainium2 kernel (final): PSROIPool, 8-core SPMD data-parallel over ROIs.
bf16 feat/masks; PE y-mask matmuls -> PSUM (512-aligned per-c regions);
ScalarE casts PSUM->SBUF bf16 (unlocks DVE 2x); DVE w-mask mul at 2x +
bf16 add-tree + fp32 reduce, with tree/reduce ops fused across ph-pairs;
ROIs sorted by pooling-window start for per-tile x-windows
([0,56) / [8,64)); ph-major feat layout DMAed in 7 chunks. Each core's 256 ROIs are sorted by
their pooling x-window start (ws0); tile 0 holds the left half (all bins
within x in [0,56)), tile 1 the right half (bins within [8,64)). Both
offsets are compile-time constants shared by all cores, so stage A/B
process 56 of 64 x-columns (12.5% less PE/ACT/DVE work). Host asserts
the windows cover every ROI and falls back to full-width otherwise.
"""

import numpy as np
import ml_dtypes

import concourse.bass as bass
import concourse.bacc as bacc
import concourse.mybir as mybir
import concourse.tile as tile
from concourse.bass_utils import run_bass_kernel_spmd
from contextlib import ExitStack

N_IMG = 8
OD = 5
GS = 7
C = OD * GS * GS  # 245
H = W = 64
R = 2048
SS = 1.0 / 16.0
N_CORES = 8
F32 = mybir.dt.float32
BF16 = mybir.dt.bfloat16
NPBF16 = ml_dtypes.bfloat16

_NC_CACHE: dict = {}


def _build_nc(rt: int, reps: int = 1, wcfg=((0, 64), (0, 64))):
    assert rt == 2, "pair pipeline assumes rt == 2"
    nc = bacc.Bacc()
    chx = C * W  # 15680
    WW = wcfg[0][1] - wcfg[0][0]  # window width (same for both tiles)
    assert all(b - a == WW for a, b in wcfg)
    SEG = OD * GS * WW  # per (t, ph)

    feat2 = nc.declare_dram_parameter("feat2", [128, chx], BF16, isOutput=False)
    mh = nc.declare_dram_parameter("mh", [128, rt * GS * 128], BF16, isOutput=False)
    mwr = nc.declare_dram_parameter("mwr", [128, rt * OD * GS * WW], BF16, isOutput=False)
    outp = nc.declare_dram_parameter("out", [128, rt * C], F32, isOutput=True)

    with tile.TileContext(nc) as tc:
        with ExitStack() as ctx:
            pool = ctx.enter_context(tc.tile_pool(name="sb", bufs=1 if reps == 1 else 2))
            stp = ctx.enter_context(tc.tile_pool(name="stg", bufs=2))
            prp = ctx.enter_context(tc.tile_pool(name="prd", bufs=2))
            trp = ctx.enter_context(tc.tile_pool(name="tr", bufs=2))
            psp = ctx.enter_context(
                tc.tile_pool(name="ps", bufs=4, space=bass.MemorySpace.PSUM)
            )

            for _rep in range(reps):
                featT = pool.tile([128, chx], BF16, tag="feat")
                PHW = OD * GS * W  # 2240 cols per ph
                for phc in range(GS):
                    nc.sync.dma_start(
                        featT[:, phc * PHW : (phc + 1) * PHW],
                        feat2[:, phc * PHW : (phc + 1) * PHW],
                    )
                mhT = pool.tile([128, rt * GS * 128], BF16, tag="mh")
                nc.sync.dma_start(mhT[:], mh[:])
                mwT = pool.tile([128, rt * OD * GS * WW], BF16, tag="mw")
                nc.sync.dma_start(mwT[:], mwr[:])
                outT = pool.tile([128, rt * C], F32, tag="out")

                featv = featT[:].rearrange(
                    "p (g c q x) -> p g c q x", g=GS, c=OD, q=GS
                )
                outv = outT[:].rearrange("p (t c h w) -> p t c h w", t=rt, c=OD, h=GS)

                for php in ((0, 1), (2, 3), (4, 5), (6,)):
                    nj = len(php)
                    stg = stp.tile([128, nj * rt * SEG], BF16, tag="stg")
                    stgv = stg[:].rearrange(
                        "p (j t c q x) -> p j t c q x", j=nj, t=rt, c=OD, q=GS
                    )
                    for j, ph in enumerate(php):
                        for t in range(rt):
                            x0 = wcfg[t][0]
                            k = t * GS + ph
                            lhs = mhT[:, k * 128 : (k + 1) * 128]
                            for c0 in range(0, OD, 2):
                                ncc = min(2, OD - c0)
                                ps = psp.tile([128, 1024], F32, tag="ps")
                                psv = ps[:].rearrange("p (i b) -> p i b", b=512)
                                for i in range(ncc):
                                    c = c0 + i
                                    rhs = featv[:, ph, c, :, x0 : x0 + WW]
                                    nc.tensor.matmul(
                                        psv[:, i, 0 : GS * WW], lhs, rhs,
                                        start=True, stop=True,
                                    )
                                nc.scalar.copy(
                                    stgv[:, j, t, c0 : c0 + ncc, :, :].rearrange(
                                        "p c q x -> p c (q x)"
                                    ),
                                    psv[:, 0:ncc, 0 : GS * WW],
                                )
                    prod = prp.tile([128, nj * rt * SEG], BF16, tag="prd")
                    for j in range(nj):
                        nc.vector.tensor_mul(
                            prod[:, j * rt * SEG : (j + 1) * rt * SEG],
                            stg[:, j * rt * SEG : (j + 1) * rt * SEG],
                            mwT[:],
                        )
                    pv = prod[:].rearrange("p (s x) -> p s x", x=WW)
                    nseg = nj * rt * OD * GS
                    h1 = WW // 2
                    t1 = trp.tile([128, nseg * h1], BF16, tag="t1")
                    t1v = t1[:].rearrange("p (s x) -> p s x", x=h1)
                    nc.vector.tensor_add(t1v, pv[:, :, 0:h1], pv[:, :, h1:WW])
                    h2 = h1 // 2
                    if h2 == 14:
                        # L2 writes its two 7-wide halves 8 apart (16-wide
                        # padded segments) so L3's operands stay 4B-aligned
                        # and keep the 2x DVE mode; the reduce then runs on
                        # 7-wide instead of 14-wide segments.
                        t2 = trp.tile([128, nseg * 16], BF16, tag="t2")
                        t2v = t2[:].rearrange("p (s h x) -> p s h x", h=2, x=8)
                        nc.vector.tensor_add(
                            t2v[:, :, :, 0:7],
                            t1v[:, :, 0:h2].rearrange("p s (h x) -> p s h x", h=2),
                            t1v[:, :, h2:h1].rearrange("p s (h x) -> p s h x", h=2),
                        )
                        t3 = trp.tile([128, nseg * 7], BF16, tag="t3")
                        t3v = t3[:].rearrange("p (s x) -> p s x", x=7)
                        nc.vector.tensor_add(
                            t3v, t2v[:, :, 0, 0:7], t2v[:, :, 1, 0:7]
                        )
                        red_in = t3v
                        hr = 7
                    else:
                        t2 = trp.tile([128, nseg * h2], BF16, tag="t2")
                        t2v = t2[:].rearrange("p (s x) -> p s x", x=h2)
                        nc.vector.tensor_add(t2v, t1v[:, :, 0:h2], t1v[:, :, h2:h1])
                        if h2 % 2 == 0 and (h2 // 2) % 2 == 0:
                            h3 = h2 // 2
                            t3 = trp.tile([128, nseg * h3], BF16, tag="t3")
                            t3v = t3[:].rearrange("p (s x) -> p s x", x=h3)
                            nc.vector.tensor_add(t3v, t2v[:, :, 0:h3], t2v[:, :, h3:h2])
                            red_in = t3v
                            hr = h3
                        else:
                            red_in = t2v
                            hr = h2
                    nc.vector.reduce_sum(
                        outv[:, :, :, php[0] : php[0] + nj, :].rearrange(
                            "p t c j q -> p j t c q"
                        ),
                        red_in.rearrange(
                            "p (j t c q) x -> p j t c q x", j=nj, t=rt, c=OD, q=GS
                        ),
                        axis=mybir.AxisListType.X,
                    )
                nc.sync.dma_start(outp[:], outT[:])

    nc.finalize()
    return nc


def _get_nc(rt: int, reps: int = 1, wcfg=((0, 64), (0, 64))):
    key = (rt, reps, wcfg)
    if key not in _NC_CACHE:
        _NC_CACHE[key] = _build_nc(rt, reps, wcfg)
    return _NC_CACHE[key]


def _bin_bounds(rois: np.ndarray):
    f = np.float32
    rois = rois.astype(f)
    xs = np.round(rois[:, 1]) * f(SS)
    ys = np.round(rois[:, 2]) * f(SS)
    xe = np.round(rois[:, 3] + f(1.0)) * f(SS)
    ye = np.round(rois[:, 4] + f(1.0)) * f(SS)
    roi_w = np.maximum(xe - xs, f(0.1))
    roi_h = np.maximum(ye - ys, f(0.1))
    inv_gs = f(1.0) / f(GS)
    bin_w = (roi_w * inv_gs).astype(f)
    bin_h = (roi_h * inv_gs).astype(f)
    pidx = np.arange(GS, dtype=f)
    hstart = np.clip(np.floor(pidx[None, :] * bin_h[:, None] + ys[:, None]), 0, H)
    hend = np.clip(np.ceil((pidx[None, :] + f(1.0)) * bin_h[:, None] + ys[:, None]), 0, H)
    wstart = np.clip(np.floor(pidx[None, :] * bin_w[:, None] + xs[:, None]), 0, W)
    wend = np.clip(np.ceil((pidx[None, :] + f(1.0)) * bin_w[:, None] + xs[:, None]), 0, W)
    return hstart, hend, wstart, wend


def _shard(rois: np.ndarray):
    batch = rois[:, 0].astype(np.int32)
    order = np.argsort(batch, kind="stable")
    if R % N_CORES == 0:
        chunks = [order[i * (R // N_CORES) : (i + 1) * (R // N_CORES)] for i in range(N_CORES)]
        if all(len(np.unique(batch[c])) <= 2 for c in chunks):
            return chunks, (R // N_CORES + 127) // 128, batch, chunks
    chunks = [np.nonzero(batch == i)[0] for i in range(N_CORES)]
    maxc = max(len(c) for c in chunks)
    rt = (maxc + 127) // 128
    return chunks, rt, batch, chunks


def _sort_and_windows(rois, chunks):
    """Sort each core's ROIs by window start; pick per-tile x-windows.

    Returns (sorted chunks, wcfg) where wcfg = ((x0_t0, x1_t0), (x0_t1,
    x1_t1)) compile-time windows shared by all cores, or full-width if the
    data doesn't fit the windows."""
    hs, he, ws, we = _bin_bounds(rois)
    ws0 = ws[:, 0]
    we6 = we[:, 6]
    schunks = [c[np.argsort(ws0[c], kind="stable")] for c in chunks]
    wcfg = ((0, 56), (8, 64))
    ok = True
    for c in schunks:
        t0, t1 = c[:128], c[128:]
        if len(t0) and we6[t0].max() > 56:
            ok = False
        if len(t1) and ws0[t1].min() < 8:
            ok = False
    if not ok:
        wcfg = ((0, 64), (0, 64))
    return schunks, wcfg


def _host_inputs(feat, rois, chunks, rt, batch, wcfg):
    hs, he, ws, we = _bin_bounds(rois)
    cnt_h = (he - hs).astype(np.float32)
    cnt_w = (we - ws).astype(np.float32)
    inv_h = np.where(cnt_h > 0, np.float32(1.0) / np.maximum(cnt_h, 1), 0).astype(np.float32)
    inv_w = np.where(cnt_w > 0, np.float32(1.0) / np.maximum(cnt_w, 1), 0).astype(np.float32)

    yi = np.arange(H, dtype=np.float32)
    xi = np.arange(W, dtype=np.float32)
    mask_h = ((yi[None, None, :] >= hs[:, :, None]) & (yi[None, None, :] < he[:, :, None])).astype(np.float32)
    mask_h *= inv_h[:, :, None]
    mask_w = ((xi[None, None, :] >= ws[:, :, None]) & (xi[None, None, :] < we[:, :, None])).astype(np.float32)
    mask_w *= inv_w[:, :, None]

    in_maps = []
    for core in range(N_CORES):
        idx = chunks[core]
        n_r = len(idx)
        imgs = np.unique(batch[idx])
        assert len(imgs) <= 2, f"core {core} spans {len(imgs)} images"
        iA = int(imgs[0])
        iB = int(imgs[1]) if len(imgs) > 1 else iA
        slot = (batch[idx] == iB).astype(np.int64) if iB != iA else np.zeros(n_r, np.int64)

        fpair = feat[[iA, iB]]  # [2, C, H, W] with C = (c, ph, pw)
        f6 = fpair.reshape(2, OD, GS, GS, H, W)
        # -> [(slot, y), (ph, c, pw, x)]
        feat2 = np.ascontiguousarray(
            f6.transpose(0, 4, 2, 1, 3, 5).reshape(128, C * W)
        ).astype(NPBF16)

        rr = np.arange(n_r)
        rt_idx = rr // 128
        rp_idx = rr % 128

        mh_t = np.zeros((rt, 128, 2, GS, H), np.float32)
        mh_t[rt_idx, rp_idx, slot] = mask_h[idx]
        mh_host = np.ascontiguousarray(
            mh_t.transpose(2, 4, 0, 3, 1).reshape(128, rt * GS * 128)
        ).astype(NPBF16)

        WW = wcfg[0][1] - wcfg[0][0]
        mw_t = np.zeros((rt, 128, GS, WW), np.float32)
        for t in range(rt):
            x0, x1 = wcfg[t]
            sel = rt_idx == t
            mw_t[t, rp_idx[sel]] = mask_w[idx[sel]][:, :, x0:x1]
        mwr_host = np.ascontiguousarray(
            np.broadcast_to(
                mw_t.transpose(1, 0, 2, 3)[:, :, None, :, :], (128, rt, OD, GS, WW)
            ).reshape(128, rt * OD * GS * WW)
        ).astype(NPBF16)

        in_maps.append({"feat2": feat2, "mh": mh_host, "mwr": mwr_host})
    return in_maps


def _run_cores(feat: np.ndarray, rois: np.ndarray, trace: bool = False, reps: int = 1):
    feat = np.ascontiguousarray(np.asarray(feat, dtype=np.float32))
    rois = np.asarray(rois, dtype=np.float32)
    assert feat.shape == (N_IMG, C, H, W), feat.shape
    assert rois.shape == (R, 5), rois.shape

    chunks, rt, batch, _ = _shard(rois)
    if rt == 2:
        chunks, wcfg = _sort_and_windows(rois, chunks)
    else:
        wcfg = ((0, 64), (0, 64))
    cap = rt * 128
    nc = _get_nc(rt, reps, wcfg)
    in_maps = _host_inputs(feat, rois, chunks, rt, batch, wcfg)

    res = run_bass_kernel_spmd(nc, in_maps, list(range(N_CORES)), trace=trace)

    out_full = np.zeros((R, OD, GS, GS), np.float32)
    for core in range(N_CORES):
        idx = chunks[core]
        o = np.asarray(res.results[core]["out"])
        o = o.reshape(128, rt, OD, GS, GS).transpose(1, 0, 2, 3, 4).reshape(cap, OD, GS, GS)
        out_full[idx] = o[: len(idx)]
    return out_full, res


def kernel(feat: np.ndarray, rois: np.ndarray) -> np.ndarray:
    out, _ = _run_cores(feat, rois, trace=False)
    return out

